# revision 4
# baseline (speedup 1.0000x reference)
"""Trainium2 Bass kernel for nn_RW_GNN (gnn_message_passing), 8 NeuronCores.

Math: the reference's P_power starts as all-ones [S,k,N] and is only ever
left-multiplied by a_sub, so it stays constant along n.  The whole model
collapses to

    c_i   = adj^T c_{i-1},  c_0 = 1          (three 2048-matvecs, exact ints)
    w_i   = 1^T a_sub^i 1                    (tiny, per-subgraph scalar)
    D_i   = S' c_i,  S'[g,n] = [gi[n]==g]/count[g]
    x[g, 32(i-1)+s] = w_i[s] * D_i[g]
    out   = relu(BN(x) @ w1 + b1) @ w2 + b2

Sharding: columns of adj (the matvec OUTPUT index) are sharded across the 8
cores, 256 each; after each matvec the 256-piece is AllGathered so every core
holds the full c_i for the next step.  The tiny D_i / theta-power / BN / MLP
epilogue is replicated on every core; core 0's output is returned.
"""

import numpy as np

N_NODES = 2048
N_CORES = 8
MBLK = N_NODES // N_CORES      # 256 columns per core
NCHUNK = N_NODES // 128        # 16 K-chunks of 128
N_SUB = 32
SIZE = 10
N_PAIRS = SIZE * (SIZE - 1) // 2   # 45
N_GRAPHS = 128
FEAT = 96
HID = 128
ODIM = 2
EPS = 1e-5

_CACHE = {}


def _build_nc():
    import concourse.bass as bass
    import concourse.bacc as bacc
    import concourse.tile as tile
    from concourse import mybir

    f32 = mybir.dt.float32
    AF = mybir.ActivationFunctionType
    ALU = mybir.AluOpType
    AX = mybir.AxisListType

    nc = bacc.Bacc("TRN2", target_bir_lowering=False, debug=False,
                   num_devices=N_CORES)

    # ---- kernel I/O ------------------------------------------------------
    adj_sh = nc.dram_tensor("adj_sh", [NCHUNK, 128, MBLK], f32,
                            kind="ExternalInput")        # per-core column block
    spt_sh = nc.dram_tensor("spt_sh", [128, NCHUNK, N_GRAPHS], f32,
                            kind="ExternalInput")        # S'^T, [p, chunk, g]
    theta2d = nc.dram_tensor("theta2d", [N_SUB, N_PAIRS], f32,
                             kind="ExternalInput")
    p2 = nc.dram_tensor("p2", [N_PAIRS, SIZE * SIZE], f32,
                        kind="ExternalInput")            # symmetric pair scatter
    ident = nc.dram_tensor("ident", [N_SUB, N_SUB], f32,
                           kind="ExternalInput")         # I_32
    params_col = nc.dram_tensor("params_col", [128, 3], f32,
                                kind="ExternalInput")    # gamma | beta | b1
    b2row = nc.dram_tensor("b2row", [1, ODIM], f32, kind="ExternalInput")
    w1_in = nc.dram_tensor("w1_in", [FEAT, HID], f32, kind="ExternalInput")
    w2_in = nc.dram_tensor("w2_in", [HID, ODIM], f32, kind="ExternalInput")
    out = nc.dram_tensor("out", [N_GRAPHS, ODIM], f32, kind="ExternalOutput")

    RG = [list(range(N_CORES))]

    with tile.TileContext(nc) as tc:
        with (
            tc.tile_pool(name="sb", bufs=1) as sb,
            tc.tile_pool(name="adjp", bufs=NCHUNK) as adjp,
            tc.tile_pool(name="ps", bufs=1, space="PSUM") as ps,
            tc.tile_pool(name="dram", bufs=1, space="DRAM") as dram,
        ):
            # ---- constant / parameter loads (off critical path) ----------
            adj_t = []
            for c in range(NCHUNK):
                t = adjp.tile([128, MBLK], f32, name=f"adj_t{c}", tag="adj_t")
                nc.sync.dma_start(out=t, in_=adj_sh[c])
                adj_t.append(t)
            spt_sb = sb.tile([128, NCHUNK, N_GRAPHS], f32)
            nc.sync.dma_start(out=spt_sb, in_=spt_sh[:, :, :])
            theta_sb = sb.tile([N_SUB, N_PAIRS], f32)
            nc.sync.dma_start(out=theta_sb, in_=theta2d[:, :])
            p2_sb = sb.tile([N_PAIRS, SIZE * SIZE], f32)
            nc.sync.dma_start(out=p2_sb, in_=p2[:, :])
            id_sb = sb.tile([N_SUB, N_SUB], f32)
            nc.sync.dma_start(out=id_sb, in_=ident[:, :])
            pc_sb = sb.tile([128, 3], f32)
            nc.sync.dma_start(out=pc_sb, in_=params_col[:, :])
            b2_sb = sb.tile([1, ODIM], f32)
            nc.sync.dma_start(out=b2_sb, in_=b2row[:, :])
            w1_sb = sb.tile([FEAT, HID], f32)
            nc.sync.dma_start(out=w1_sb, in_=w1_in[:, :])
            w2_sb = sb.tile([HID, ODIM], f32)
            nc.sync.dma_start(out=w2_sb, in_=w2_in[:, :])

            ones_col = sb.tile([128, 1], f32)
            nc.vector.memset(ones_col, 1.0)
            ones_row = sb.tile([1, HID], f32)
            nc.vector.memset(ones_row, 1.0)
            eps_t = sb.tile([FEAT, 1], f32)
            nc.vector.memset(eps_t, EPS)

            # ---- theta path: w_i = 1^T a_sub^i 1  (replicated, tiny) -----
            thT_ps = ps.tile([N_PAIRS, N_SUB], f32, tag="th")
            nc.tensor.transpose(thT_ps, theta_sb, id_sb)          # theta^T
            relu_thT = sb.tile([N_PAIRS, N_SUB], f32)
            nc.scalar.activation(relu_thT, thT_ps, AF.Relu)
            a_ps = ps.tile([N_SUB, SIZE * SIZE], f32, tag="th")
            nc.tensor.matmul(a_ps, lhsT=relu_thT, rhs=p2_sb, start=True, stop=True)
            a_sb = sb.tile([N_SUB, SIZE, SIZE], f32)
            nc.scalar.copy(a_sb.rearrange("s a b -> s (a b)"),
                           a_ps)                                   # a_sub, [32,10,10]

            w3_sb = sb.tile([N_SUB, 3], f32)
            u_prev = None
            tmp_u = sb.tile([N_SUB, SIZE, SIZE], f32)
            for i in range(3):
                u_i = sb.tile([N_SUB, SIZE, 1], f32, name=f"u_{i}", tag=f"u{i}")
                if i == 0:
                    nc.vector.reduce_sum(u_i, a_sb, axis=AX.X)
                else:
                    nc.vector.tensor_mul(
                        tmp_u, a_sb,
                        u_prev.rearrange("s b one -> s (one b)")[:, None, :]
                        .broadcast_to([N_SUB, SIZE, SIZE]))
                    nc.vector.reduce_sum(u_i, tmp_u, axis=AX.X)
                nc.vector.reduce_sum(w3_sb[:, i:i + 1],
                                     u_i.rearrange("s a one -> s (a one)"),
                                     axis=AX.X)
                u_prev = u_i

            wrow = []
            for i in range(3):
                wr_ps = ps.tile([1, N_SUB], f32, name=f"wr_ps{i}", tag="th")
                nc.tensor.transpose(wr_ps, w3_sb[:, i:i + 1], id_sb)
                wr_sb = sb.tile([1, N_SUB], f32, name=f"wr_sb{i}", tag=f"wr{i}")
                nc.scalar.copy(wr_sb, wr_ps)
                wrow.append(wr_sb)

            # ---- chained matvec + AllGather + segment pools --------------
            c_sb = []       # gathered full c_i, [128, 16] column layout
            drow = []       # D_i rows [1, 128]
            lhs_step = ones_col
            for step in range(3):
                mv_ps = ps.tile([1, MBLK], f32, name=f"mv{step}", tag="mv")
                for c in range(NCHUNK):
                    nc.tensor.matmul(mv_ps, lhsT=lhs_step[:, c:c + 1]
                                     if step > 0 else ones_col,
                                     rhs=adj_t[c],
                                     start=(c == 0), stop=(c == NCHUNK - 1))
                c_loc = sb.tile([1, MBLK], f32, name=f"c_loc{step}", tag="cloc")
                nc.scalar.copy(c_loc, mv_ps)

                cc_in = dram.tile([1, MBLK], f32, name=f"cc_in{step}")
                cc_out = dram.tile([N_CORES, MBLK], f32, name=f"cc_out{step}")
                nc.gpsimd.dma_start(out=cc_in, in_=c_loc)
                nc.gpsimd.collective_compute(
                    "AllGather", mybir.AluOpType.bypass,
                    replica_groups=RG, ins=[cc_in.opt()], outs=[cc_out.opt()])

                ct = sb.tile([NCHUNK, 128], f32, name=f"ct{step}", tag="ct")
                nc.sync.dma_start(
                    out=ct, in_=cc_out.rearrange("r (c p) -> (r c) p", p=128))
                tr_ps = ps.tile([128, NCHUNK], f32, name=f"tr{step}", tag="tr")
                nc.tensor.transpose(tr_ps, ct, id_sb[0:NCHUNK, 0:NCHUNK])
                cs = sb.tile([128, NCHUNK], f32, name=f"c_sb{step}",
                             tag=f"csb{step}")
                nc.scalar.copy(cs, tr_ps)
                c_sb.append(cs)
                lhs_step = cs

                # D_i = S' c_i  — 16 chunk-matvecs vs the gathered vector
                d_ps = ps.tile([1, N_GRAPHS], f32, name=f"d{step}", tag="d")
                for c in range(NCHUNK):
                    nc.tensor.matmul(d_ps, lhsT=cs[:, c:c + 1],
                                     rhs=spt_sb[:, c, :],
                                     start=(c == 0), stop=(c == NCHUNK - 1))
                dr = sb.tile([1, N_GRAPHS], f32, name=f"drow{step}",
                             tag=f"dr{step}")
                nc.scalar.copy(dr, d_ps)
                drow.append(dr)

            # ---- x^T = sum_i wrow_i ⊗ drow_i  (block outer products) -----
            xT_ps = ps.tile([FEAT, N_GRAPHS], f32, tag="big")
            for i in range(3):
                nc.tensor.matmul(xT_ps[32 * i:32 * (i + 1), :],
                                 lhsT=wrow[i], rhs=drow[i],
                                 start=True, stop=True)
            x_sb = sb.tile([FEAT, N_GRAPHS], f32)
            nc.scalar.copy(x_sb, xT_ps)

            # ---- BatchNorm over graphs (free axis) -----------------------
            stats = sb.tile([FEAT, 6], f32)
            nc.vector.bn_stats(out=stats, in_=x_sb)
            mv_aggr = sb.tile([FEAT, 2], f32)
            nc.vector.bn_aggr(out=mv_aggr, in_=stats)
            stdev = sb.tile([FEAT, 1], f32)
            nc.scalar.activation(stdev, mv_aggr[:, 1:2], AF.Sqrt,
                                 bias=eps_t, scale=1.0)
            invstd = sb.tile([FEAT, 1], f32)
            nc.vector.reciprocal(invstd, stdev)
            alpha = sb.tile([FEAT, 1], f32)
            nc.vector.tensor_mul(alpha, pc_sb[0:FEAT, 0:1], invstd)
            amean = sb.tile([FEAT, 1], f32)
            nc.vector.tensor_mul(amean, alpha, mv_aggr[:, 0:1])
            shift = sb.tile([FEAT, 1], f32)
            nc.vector.tensor_sub(shift, pc_sb[0:FEAT, 1:2], amean)
            xh = sb.tile([FEAT, N_GRAPHS], f32)
            nc.vector.tensor_scalar(out=xh, in0=x_sb, scalar1=alpha,
                                    scalar2=shift, op0=ALU.mult, op1=ALU.add)

            # ---- MLP -----------------------------------------------------
            h_ps = ps.tile([HID, N_GRAPHS], f32, tag="big")
            nc.tensor.matmul(h_ps, lhsT=w1_sb, rhs=xh, start=True, stop=True)
            h_sb = sb.tile([HID, N_GRAPHS], f32)
            nc.scalar.activation(h_sb, h_ps, AF.Relu,
                                 bias=pc_sb[0:HID, 2:3], scale=1.0)
            o_ps = ps.tile([N_GRAPHS, ODIM], f32, tag="big")
            nc.tensor.matmul(o_ps, lhsT=ones_row, rhs=b2_sb,
                             start=True, stop=False)
            nc.tensor.matmul(o_ps, lhsT=h_sb, rhs=w2_sb,
                             start=False, stop=True)
            o_sb = sb.tile([N_GRAPHS, ODIM], f32)
            nc.scalar.copy(o_sb, o_ps)
            nc.sync.dma_start(out=out[:, :], in_=o_sb)

    nc.compile()
    return nc


def _host_prep(adj, graph_indicator, theta, gamma, beta, w1, b1, w2, b2):
    adj = np.ascontiguousarray(adj, dtype=np.float32)
    gi = np.asarray(graph_indicator).astype(np.int64)
    counts = np.bincount(gi, minlength=N_GRAPHS).astype(np.float32)
    counts_safe = np.maximum(counts, 1.0)
    # S'^T [n, g] = (gi[n] == g) / count[g], chunk-major [16, 128, 128]
    spt = np.zeros((N_NODES, N_GRAPHS), dtype=np.float32)
    spt[np.arange(N_NODES), gi] = 1.0 / counts_safe[gi]
    spt_sh = np.ascontiguousarray(
        spt.reshape(NCHUNK, 128, N_GRAPHS).transpose(1, 0, 2))

    theta2d = np.ascontiguousarray(theta[:, :, 0].astype(np.float32))
    iu = np.triu_indices(SIZE, k=1)
    p2 = np.zeros((N_PAIRS, SIZE * SIZE), dtype=np.float32)
    for p, (i, j) in enumerate(zip(iu[0], iu[1])):
        p2[p, i * SIZE + j] = 1.0
        p2[p, j * SIZE + i] = 1.0
    ident = np.eye(N_SUB, dtype=np.float32)
    params_col = np.zeros((128, 3), dtype=np.float32)
    params_col[:FEAT, 0] = gamma
    params_col[:FEAT, 1] = beta
    params_col[:HID, 2] = b1
    b2row = np.ascontiguousarray(b2.reshape(1, ODIM).astype(np.float32))
    w1c = np.ascontiguousarray(w1.astype(np.float32))
    w2c = np.ascontiguousarray(w2.astype(np.float32))

    shared = dict(spt_sh=spt_sh, theta2d=theta2d, p2=p2, ident=ident,
                  params_col=params_col, b2row=b2row, w1_in=w1c, w2_in=w2c)
    in_maps = []
    for j in range(N_CORES):
        blk = np.ascontiguousarray(
            adj[:, j * MBLK:(j + 1) * MBLK].reshape(NCHUNK, 128, MBLK))
        in_maps.append(dict(adj_sh=blk, **shared))
    return in_maps


def kernel(**inputs) -> np.ndarray:
    from concourse.bass_utils import run_bass_kernel_spmd

    if "nc" not in _CACHE:
        _CACHE["nc"] = _build_nc()
    nc = _CACHE["nc"]
    in_maps = _host_prep(**inputs)
    res = run_bass_kernel_spmd(nc, in_maps, core_ids=list(range(N_CORES)))
    _CACHE["last_result"] = res
    return res.results[0]["out"]


# revision 5
# speedup vs baseline: 1.3952x; 1.3952x over previous
"""Trainium2 Bass kernel for nn_RW_GNN (gnn_message_passing), 8 NeuronCores.

Math: the reference's P_power starts as all-ones [S,k,N] and is only ever
left-multiplied by a_sub, so it stays constant along n.  The whole model
collapses to

    c_i   = adj^T c_{i-1},  c_0 = 1          (three 2048-matvecs, exact ints)
    w_i   = 1^T a_sub^i 1                    (tiny, per-subgraph scalar)
    D_i   = (S c_i) / counts,  S[g,n] = [gi[n]==g]
    x[g, 32(i-1)+s] = w_i[s] * D_i[g]
    out   = relu(BN(x) @ w1 + b1) @ w2 + b2

Sharding: columns of adj (the matvec OUTPUT index) are sharded across the 8
cores, 256 each; after each matvec the 256-piece is AllGathered so every core
holds the full c_i for the next step.  The tiny D_i / theta-power / BN / MLP
epilogue is replicated on every core; core 0's output is returned.

Precision: adj and the one-hot segment matrix are 0/1 -> exact in fp16; the
c vectors are integers (c1<=~40, c2<=~600 exact in fp16; c3<=~10000 is split
into an exact fp16 hi/lo pair whose two matmul passes accumulate in the same
fp32 PSUM row).  All matmul accumulation is fp32, so the c chain and segment
sums are bit-exact integer arithmetic.
"""

import numpy as np

N_NODES = 2048
N_CORES = 8
MBLK = N_NODES // N_CORES      # 256 columns per core
NCHUNK = N_NODES // 128        # 16 K-chunks of 128
N_SUB = 32
SIZE = 10
N_PAIRS = SIZE * (SIZE - 1) // 2   # 45
N_GRAPHS = 128
FEAT = 96
HID = 128
ODIM = 2
EPS = 1e-5

_CACHE = {}


def _build_nc():
    import concourse.bass as bass
    import concourse.bacc as bacc
    import concourse.tile as tile
    from concourse import mybir

    f32 = mybir.dt.float32
    f16 = mybir.dt.float16
    AF = mybir.ActivationFunctionType
    ALU = mybir.AluOpType
    AX = mybir.AxisListType

    nc = bacc.Bacc("TRN2", target_bir_lowering=False, debug=False,
                   num_devices=N_CORES)

    # ---- kernel I/O ------------------------------------------------------
    adj_sh = nc.dram_tensor("adj_sh", [NCHUNK, 128, MBLK], f16,
                            kind="ExternalInput")        # per-core column block
    spt_sh = nc.dram_tensor("spt_sh", [128, NCHUNK, N_GRAPHS], f16,
                            kind="ExternalInput")        # one-hot S^T [p, c, g]
    theta2d = nc.dram_tensor("theta2d", [N_SUB, N_PAIRS], f32,
                             kind="ExternalInput")
    p2 = nc.dram_tensor("p2", [N_PAIRS, SIZE * SIZE], f32,
                        kind="ExternalInput")            # symmetric pair scatter
    ident = nc.dram_tensor("ident", [N_SUB, N_SUB], f32,
                           kind="ExternalInput")         # I_32
    params_col = nc.dram_tensor("params_col", [128, 3], f32,
                                kind="ExternalInput")    # gamma | beta | b1
    cinv_row = nc.dram_tensor("cinv_row", [1, N_GRAPHS], f32,
                              kind="ExternalInput")      # 1/counts
    b2row = nc.dram_tensor("b2row", [1, ODIM], f32, kind="ExternalInput")
    w1_in = nc.dram_tensor("w1_in", [FEAT, HID], f32, kind="ExternalInput")
    w2_in = nc.dram_tensor("w2_in", [HID, ODIM], f32, kind="ExternalInput")
    out = nc.dram_tensor("out", [N_GRAPHS, ODIM], f32, kind="ExternalOutput")

    RG = [list(range(N_CORES))]

    with tile.TileContext(nc) as tc:
        with (
            tc.tile_pool(name="sb", bufs=1) as sb,
            tc.tile_pool(name="adjp", bufs=NCHUNK) as adjp,
            tc.tile_pool(name="ps", bufs=1, space="PSUM") as ps,
            tc.tile_pool(name="dram", bufs=1, space="DRAM") as dram,
        ):
            # ---- adj chunks first: they gate step 1 ----------------------
            adj_t = []
            for c in range(NCHUNK):
                t = adjp.tile([128, MBLK], f16, name=f"adj_t{c}", tag="adj_t")
                nc.sync.dma_start(out=t, in_=adj_sh[c])
                adj_t.append(t)

            ones_col = sb.tile([128, 1], f16)
            nc.vector.memset(ones_col, 1.0)
            ones_row = sb.tile([1, HID], f32)
            nc.vector.memset(ones_row, 1.0)
            eps_t = sb.tile([FEAT, 1], f32)
            nc.vector.memset(eps_t, EPS)

            # ---- small parameters (gpsimd queues; off critical path) -----
            theta_sb = sb.tile([N_SUB, N_PAIRS], f32)
            nc.gpsimd.dma_start(out=theta_sb, in_=theta2d[:, :])
            p2_sb = sb.tile([N_PAIRS, SIZE * SIZE], f32)
            nc.gpsimd.dma_start(out=p2_sb, in_=p2[:, :])
            id_sb = sb.tile([N_SUB, N_SUB], f32)
            nc.gpsimd.dma_start(out=id_sb, in_=ident[:, :])
            pc_sb = sb.tile([128, 3], f32)
            nc.gpsimd.dma_start(out=pc_sb, in_=params_col[:, :])
            cinv_sb = sb.tile([1, N_GRAPHS], f32)
            nc.gpsimd.dma_start(out=cinv_sb, in_=cinv_row[:, :])
            b2_sb = sb.tile([1, ODIM], f32)
            nc.gpsimd.dma_start(out=b2_sb, in_=b2row[:, :])
            w1_sb = sb.tile([FEAT, HID], f32)
            nc.gpsimd.dma_start(out=w1_sb, in_=w1_in[:, :])
            w2_sb = sb.tile([HID, ODIM], f32)
            nc.gpsimd.dma_start(out=w2_sb, in_=w2_in[:, :])
            spt_sb = sb.tile([128, NCHUNK, N_GRAPHS], f16)
            nc.gpsimd.dma_start(out=spt_sb, in_=spt_sh[:, :, :])

            # ---- theta path: w_i = 1^T a_sub^i 1 (tiny, replicated) ------
            thT_ps = ps.tile([N_PAIRS, N_SUB], f32, tag="th")
            nc.tensor.transpose(thT_ps, theta_sb, id_sb)          # theta^T
            relu_thT = sb.tile([N_PAIRS, N_SUB], f32)
            nc.scalar.activation(relu_thT, thT_ps, AF.Relu)
            a_ps = ps.tile([N_SUB, SIZE * SIZE], f32, tag="th")
            nc.tensor.matmul(a_ps, lhsT=relu_thT, rhs=p2_sb, start=True, stop=True)
            a_sb = sb.tile([N_SUB, SIZE, SIZE], f32)
            nc.scalar.copy(a_sb.rearrange("s a b -> s (a b)"), a_ps)

            w3_sb = sb.tile([N_SUB, 3], f32)
            u_prev = None
            tmp_u = sb.tile([N_SUB, SIZE, SIZE], f32)
            for i in range(3):
                u_i = sb.tile([N_SUB, SIZE, 1], f32, name=f"u_{i}", tag=f"u{i}")
                if i == 0:
                    nc.vector.reduce_sum(u_i, a_sb, axis=AX.X)
                else:
                    nc.vector.tensor_mul(
                        tmp_u, a_sb,
                        u_prev.rearrange("s b one -> s (one b)")[:, None, :]
                        .broadcast_to([N_SUB, SIZE, SIZE]))
                    nc.vector.reduce_sum(u_i, tmp_u, axis=AX.X)
                nc.vector.reduce_sum(w3_sb[:, i:i + 1],
                                     u_i.rearrange("s a one -> s (a one)"),
                                     axis=AX.X)
                u_prev = u_i

            wrow = []
            for i in range(3):
                wr_ps = ps.tile([1, N_SUB], f32, name=f"wr_ps{i}", tag="th")
                nc.tensor.transpose(wr_ps, w3_sb[:, i:i + 1], id_sb)
                wr_sb = sb.tile([1, N_SUB], f32, name=f"wr_sb{i}", tag=f"wr{i}")
                nc.scalar.copy(wr_sb, wr_ps)
                wrow.append(wr_sb)

            # ---- chained matvec + AllGather + segment pools --------------
            # lhs columns per step: list of fp16 [128,16] tiles (hi/lo passes)
            drow = []
            lhs_cols = None
            for step in range(3):
                mv_ps = ps.tile([1, MBLK], f32, name=f"mv{step}", tag="mv")
                if step == 0:
                    passes = [ones_col]
                    for c in range(NCHUNK):
                        nc.tensor.matmul(mv_ps, lhsT=ones_col, rhs=adj_t[c],
                                         start=(c == 0), stop=(c == NCHUNK - 1))
                else:
                    n_mm = len(lhs_cols) * NCHUNK
                    k = 0
                    for col in lhs_cols:
                        for c in range(NCHUNK):
                            nc.tensor.matmul(mv_ps, lhsT=col[:, c:c + 1],
                                             rhs=adj_t[c],
                                             start=(k == 0), stop=(k == n_mm - 1))
                            k += 1
                c_loc = sb.tile([1, MBLK], f32, name=f"c_loc{step}", tag="cloc")
                nc.scalar.copy(c_loc, mv_ps)

                cc_in = dram.tile([1, MBLK], f32, name=f"cc_in{step}")
                cc_out = dram.tile([N_CORES, MBLK], f32, name=f"cc_out{step}")
                nc.gpsimd.dma_start(out=cc_in, in_=c_loc)
                nc.gpsimd.collective_compute(
                    "AllGather", mybir.AluOpType.bypass,
                    replica_groups=RG, ins=[cc_in.opt()], outs=[cc_out.opt()])

                ct = sb.tile([NCHUNK, 128], f32, name=f"ct{step}", tag="ct")
                nc.sync.dma_start(
                    out=ct, in_=cc_out.rearrange("r (c p) -> (r c) p", p=128))
                tr_ps = ps.tile([128, NCHUNK], f32, name=f"tr{step}", tag="tr")
                nc.tensor.transpose(tr_ps, ct, id_sb[0:NCHUNK, 0:NCHUNK])

                # fp16 lhs columns for the next matvec / this D_i
                hi = sb.tile([128, NCHUNK], f16, name=f"hi{step}", tag=f"hi{step}")
                nc.vector.tensor_copy(hi, tr_ps)
                if step < 2:
                    # c1, c2 are <=~600: fp16-exact, single pass
                    cols = [hi]
                else:
                    # c3 <=~10000: hi/lo split, both fp16-exact
                    lo_f = sb.tile([128, NCHUNK], f32, name="lo_f")
                    nc.vector.tensor_sub(lo_f, tr_ps, hi)
                    lo = sb.tile([128, NCHUNK], f16, name=f"lo{step}",
                                 tag=f"lo{step}")
                    nc.vector.tensor_copy(lo, lo_f)
                    cols = [hi, lo]
                lhs_cols = cols

                # D_i = (S c_i) / counts — chunked matvec vs gathered vector
                d_ps = ps.tile([1, N_GRAPHS], f32, name=f"d{step}", tag="d")
                n_mm = len(cols) * NCHUNK
                k = 0
                for col in cols:
                    for c in range(NCHUNK):
                        nc.tensor.matmul(d_ps, lhsT=col[:, c:c + 1],
                                         rhs=spt_sb[:, c, :],
                                         start=(k == 0), stop=(k == n_mm - 1))
                        k += 1
                dr = sb.tile([1, N_GRAPHS], f32, name=f"drow{step}",
                             tag=f"dr{step}")
                nc.vector.tensor_mul(dr, d_ps, cinv_sb)
                drow.append(dr)

            # ---- x^T = sum_i wrow_i (x) drow_i (block outer products) ----
            xT_ps = ps.tile([FEAT, N_GRAPHS], f32, tag="big")
            for i in range(3):
                nc.tensor.matmul(xT_ps[32 * i:32 * (i + 1), :],
                                 lhsT=wrow[i], rhs=drow[i],
                                 start=True, stop=True)
            x_sb = sb.tile([FEAT, N_GRAPHS], f32)
            nc.scalar.copy(x_sb, xT_ps)

            # ---- BatchNorm over graphs (free axis) -----------------------
            stats = sb.tile([FEAT, 6], f32)
            nc.vector.bn_stats(out=stats, in_=x_sb)
            mv_aggr = sb.tile([FEAT, 2], f32)
            nc.vector.bn_aggr(out=mv_aggr, in_=stats)
            stdev = sb.tile([FEAT, 1], f32)
            nc.scalar.activation(stdev, mv_aggr[:, 1:2], AF.Sqrt,
                                 bias=eps_t, scale=1.0)
            invstd = sb.tile([FEAT, 1], f32)
            nc.vector.reciprocal(invstd, stdev)
            alpha = sb.tile([FEAT, 1], f32)
            nc.vector.tensor_mul(alpha, pc_sb[0:FEAT, 0:1], invstd)
            amean = sb.tile([FEAT, 1], f32)
            nc.vector.tensor_mul(amean, alpha, mv_aggr[:, 0:1])
            shift = sb.tile([FEAT, 1], f32)
            nc.vector.tensor_sub(shift, pc_sb[0:FEAT, 1:2], amean)
            xh = sb.tile([FEAT, N_GRAPHS], f32)
            nc.vector.tensor_scalar(out=xh, in0=x_sb, scalar1=alpha,
                                    scalar2=shift, op0=ALU.mult, op1=ALU.add)

            # ---- MLP -----------------------------------------------------
            h_ps = ps.tile([HID, N_GRAPHS], f32, tag="big")
            nc.tensor.matmul(h_ps, lhsT=w1_sb, rhs=xh, start=True, stop=True)
            h_sb = sb.tile([HID, N_GRAPHS], f32)
            nc.scalar.activation(h_sb, h_ps, AF.Relu,
                                 bias=pc_sb[0:HID, 2:3], scale=1.0)
            o_ps = ps.tile([N_GRAPHS, ODIM], f32, tag="big")
            nc.tensor.matmul(o_ps, lhsT=ones_row, rhs=b2_sb,
                             start=True, stop=False)
            nc.tensor.matmul(o_ps, lhsT=h_sb, rhs=w2_sb,
                             start=False, stop=True)
            o_sb = sb.tile([N_GRAPHS, ODIM], f32)
            nc.scalar.copy(o_sb, o_ps)
            nc.sync.dma_start(out=out[:, :], in_=o_sb)

    nc.compile()
    return nc


def _host_prep(adj, graph_indicator, theta, gamma, beta, w1, b1, w2, b2):
    import ml_dtypes
    f16 = np.float16
    adj = np.ascontiguousarray(adj, dtype=np.float32)
    gi = np.asarray(graph_indicator).astype(np.int64)
    counts = np.bincount(gi, minlength=N_GRAPHS).astype(np.float32)
    counts_safe = np.maximum(counts, 1.0)
    # one-hot S^T [n, g], chunk-major -> [p, chunk, g], fp16 (0/1 exact)
    spt = np.zeros((N_NODES, N_GRAPHS), dtype=f16)
    spt[np.arange(N_NODES), gi] = 1.0
    spt_sh = np.ascontiguousarray(
        spt.reshape(NCHUNK, 128, N_GRAPHS).transpose(1, 0, 2))

    theta2d = np.ascontiguousarray(theta[:, :, 0].astype(np.float32))
    iu = np.triu_indices(SIZE, k=1)
    p2 = np.zeros((N_PAIRS, SIZE * SIZE), dtype=np.float32)
    for p, (i, j) in enumerate(zip(iu[0], iu[1])):
        p2[p, i * SIZE + j] = 1.0
        p2[p, j * SIZE + i] = 1.0
    ident = np.eye(N_SUB, dtype=np.float32)
    params_col = np.zeros((128, 3), dtype=np.float32)
    params_col[:FEAT, 0] = gamma
    params_col[:FEAT, 1] = beta
    params_col[:HID, 2] = b1
    cinv = np.ascontiguousarray((1.0 / counts_safe).reshape(1, N_GRAPHS))
    b2row = np.ascontiguousarray(b2.reshape(1, ODIM).astype(np.float32))
    w1c = np.ascontiguousarray(w1.astype(np.float32))
    w2c = np.ascontiguousarray(w2.astype(np.float32))

    shared = dict(spt_sh=spt_sh, theta2d=theta2d, p2=p2, ident=ident,
                  params_col=params_col, cinv_row=cinv, b2row=b2row,
                  w1_in=w1c, w2_in=w2c)
    in_maps = []
    for j in range(N_CORES):
        blk = np.ascontiguousarray(
            adj[:, j * MBLK:(j + 1) * MBLK].reshape(NCHUNK, 128, MBLK)
            .astype(f16))
        in_maps.append(dict(adj_sh=blk, **shared))
    return in_maps


def kernel(**inputs) -> np.ndarray:
    from concourse.bass_utils import run_bass_kernel_spmd

    if "nc" not in _CACHE:
        _CACHE["nc"] = _build_nc()
    nc = _CACHE["nc"]
    in_maps = _host_prep(**inputs)
    res = run_bass_kernel_spmd(nc, in_maps, core_ids=list(range(N_CORES)))
    _CACHE["last_result"] = res
    return res.results[0]["out"]
